# revision 4
# baseline (speedup 1.0000x reference)
"""Trainium2 Bass kernel for nn_Aggregator (segment_reduce).

Math: the reference's gather+einsum collapses algebraically:
  out[b, o] = sum_p sum_k x[b, p, indices[k]] * Wx[o, p] + const[o]
            = sum_p (sum_v count[v] * x[b, p, v]) * Wx[o, p] + const[o]
where count = histogram of `indices` and const = index-path output + K*b_aggre
(both batch-independent, computed on host from the tiny weights).

Device pipeline per core (B_loc = 2048 rows):
  1. DMA x tiles [128, G, 4704] (natural layout, batch on partitions)
  2. DVE: weighted view-reduction via fused scalar_tensor_tensor
     (acc = x[:,:, :, v] * count[v] + acc), one op per distinct view
  3. PE: transpose x_summed [128, 197] -> [197, 128] chunks (ones column
     appended so the const row folds into the stage-2 contraction)
  4. PE: matmul [p=128|69, b=128]^T @ WxT[p, o] accumulating into PSUM
  5. ACT: PSUM -> SBUF copies; DMA out [128, 1024] tiles

Sharding: batch dim split 8 ways (2048 rows/core); small weights replicated.
"""

import numpy as np

import concourse.bass as bass
import concourse.mybir as mybir
from concourse import bacc
from concourse.tile import TileContext
from concourse.bass_utils import run_bass_kernel_spmd

N_CORES = 8
B_TOTAL = 16384
B_LOC = B_TOTAL // N_CORES  # 2048
P = 196  # 14*14 spatial features
V = 24  # views
F = P * V  # 4704 features per batch row
OUT = 1024
TILE = 128
G = 4  # 128-row tiles per group (stage-1 ops amortize instr overhead over G)
N_TILES = B_LOC // TILE  # 16
N_GROUPS = N_TILES // G  # 4
P2 = P - 128 + 1  # second contraction chunk: p 128..195 plus the ones row
PW = 512  # PSUM half width (max fp32 moving cols)

_cache = {}
last_results = None  # BassKernelResults of the most recent run (for test harness)


def _build_program(count_items):
    """count_items: tuple of (view, count) with count > 0."""
    nc = bacc.Bacc(
        "TRN2",
        target_bir_lowering=False,
        debug=False,
        num_devices=N_CORES,
    )
    f32 = mybir.dt.float32
    x_in = nc.declare_dram_parameter("x_in", [B_LOC, F], f32, isOutput=False)
    wxa = nc.declare_dram_parameter("wxa", [128, OUT], f32, isOutput=False)
    wxb = nc.declare_dram_parameter("wxb", [P2, OUT], f32, isOutput=False)
    ident = nc.declare_dram_parameter("ident", [128, 128], f32, isOutput=False)
    out = nc.declare_dram_parameter("out", [B_LOC, OUT], f32, isOutput=True)

    with TileContext(nc) as tc:
        with (
            tc.tile_pool(name="consts", bufs=1) as cpool,
            tc.tile_pool(name="x", bufs=2) as xpool,
            tc.tile_pool(name="acc", bufs=2) as apool,
            tc.tile_pool(name="lhs", bufs=3) as lpool,
            tc.tile_pool(name="outs", bufs=2) as opool,
            tc.tile_pool(name="pt", bufs=2, space="PSUM") as ptpool,
            tc.tile_pool(name="po", bufs=2, space="PSUM") as popool,
        ):
            wxa_t = cpool.tile([128, OUT], f32)
            nc.sync.dma_start(out=wxa_t[:], in_=wxa[:])
            wxb_t = cpool.tile([P2, OUT], f32)
            nc.sync.dma_start(out=wxb_t[:], in_=wxb[:])
            id_t = cpool.tile([128, 128], f32)
            nc.sync.dma_start(out=id_t[:], in_=ident[:])

            for grp in range(N_GROUPS):
                rows = slice(grp * G * TILE, (grp + 1) * G * TILE)
                xt = xpool.tile([TILE, G, F], f32)
                nc.sync.dma_start(
                    out=xt[:],
                    in_=x_in[rows, :].rearrange("(g p) f -> p g f", p=TILE),
                )

                acc = apool.tile([TILE, G, P + 1], f32)
                # ones column: folds const into the stage-2 contraction
                nc.vector.memset(acc[:, :, P : P + 1], 1.0)
                xv = xt.rearrange("p g (q v) -> p g q v", v=V)
                accs = acc[:, :, 0:P]
                for i, (v, c) in enumerate(count_items):
                    xs = xv[:, :, :, v]
                    if i == 0:
                        nc.vector.tensor_scalar_mul(accs, xs, float(c))
                    else:
                        nc.vector.scalar_tensor_tensor(
                            out=accs,
                            in0=xs,
                            scalar=float(c),
                            in1=accs,
                            op0=mybir.AluOpType.mult,
                            op1=mybir.AluOpType.add,
                        )

                ot = opool.tile([TILE, G, OUT], f32)
                for g in range(G):
                    pt1 = ptpool.tile([128, 128], f32, tag="pt1")
                    nc.tensor.transpose(pt1[:], acc[:, g, 0:128], id_t[:])
                    pt2 = ptpool.tile([P2, 128], f32, tag="pt2")
                    nc.tensor.transpose(pt2[:], acc[:, g, 128 : P + 1], id_t[:])
                    l1 = lpool.tile([128, 128], f32, tag="l1")
                    nc.scalar.copy(l1[:], pt1[:])
                    l2 = lpool.tile([P2, 128], f32, tag="l2")
                    nc.scalar.copy(l2[:], pt2[:])
                    po = popool.tile([128, OUT], f32)
                    for h in range(2):
                        cols = slice(h * PW, (h + 1) * PW)
                        nc.tensor.matmul(
                            po[:, cols], l1[:], wxa_t[:, cols], start=True, stop=False
                        )
                        nc.tensor.matmul(
                            po[:, cols], l2[:], wxb_t[:, cols], start=False, stop=True
                        )
                    nc.scalar.copy(ot[:, g, :], po[:])

                nc.sync.dma_start(
                    out=out[rows, :].rearrange("(g p) f -> p g f", p=TILE),
                    in_=ot[:],
                )
    nc.finalize()
    return nc


def kernel(x, indices, W_idx, b_idx, W_aggre, b_aggre, **run_kwargs):
    global last_results
    x = np.ascontiguousarray(np.asarray(x, dtype=np.float32)).reshape(B_TOTAL, F)
    idx = np.asarray(indices).astype(np.int64).ravel()
    W_idx = np.asarray(W_idx, dtype=np.float32)
    b_idx = np.asarray(b_idx, dtype=np.float32)
    W_aggre = np.asarray(W_aggre, dtype=np.float32)
    b_aggre = np.asarray(b_aggre, dtype=np.float32)

    count = np.bincount(idx, minlength=V)
    count_items = tuple((int(v), float(count[v])) for v in range(V) if count[v] != 0)

    # batch-independent index path, on host (tiny: [24, 196] scale)
    z = W_idx[:, idx].T + b_idx  # [K, 196]
    feats = np.where(z > 0, z, np.float32(0.2) * z).astype(np.float32)
    Wi = W_aggre[:, P:]
    const = feats.sum(0, dtype=np.float32) @ Wi.T + np.float32(len(idx)) * b_aggre

    WxT = np.ascontiguousarray(W_aggre[:, :P].T)  # [196, 1024]
    wxa_np = np.ascontiguousarray(WxT[:128])
    wxb_np = np.ascontiguousarray(
        np.concatenate([WxT[128:], const[None, :].astype(np.float32)], axis=0)
    )
    ident_np = np.eye(128, dtype=np.float32)

    nc = _cache.get(count_items)
    if nc is None:
        nc = _build_program(count_items)
        _cache[count_items] = nc

    in_maps = [
        {
            "x_in": np.ascontiguousarray(x[i * B_LOC : (i + 1) * B_LOC]),
            "wxa": wxa_np,
            "wxb": wxb_np,
            "ident": ident_np,
        }
        for i in range(N_CORES)
    ]
    res = run_bass_kernel_spmd(nc, in_maps, core_ids=list(range(N_CORES)), **run_kwargs)
    last_results = res
    return np.concatenate([res.results[i]["out"] for i in range(N_CORES)], axis=0)


# revision 13
# speedup vs baseline: 1.1759x; 1.1759x over previous
"""Trainium2 Bass kernel for nn_Aggregator (segment_reduce).

Math: the reference's gather+einsum collapses algebraically:
  out[b, o] = sum_p sum_k x[b, p, indices[k]] * Wx[o, p] + const[o]
            = sum_p (sum_v count[v] * x[b, p, v]) * Wx[o, p] + const[o]
where count = histogram of `indices` and const = index-path output + K*b_aggre
(both batch-independent, computed on host from the tiny weights).

Device pipeline per core (B_loc = 2048 rows):
  1. DMA x tiles [128, G, 4704] (natural layout, batch on partitions)
  2. DVE: weighted view-reduction via fused scalar_tensor_tensor
     (acc = x[:,:, :, v] * count[v] + acc), one op per distinct view
  3. PE: transpose x_summed [128, 197] -> [197, 128] chunks (ones column
     appended so the const row folds into the stage-2 contraction)
  4. PE: matmul [p=128|69, b=128]^T @ WxT[p, o] accumulating into PSUM
  5. ACT: PSUM -> SBUF copies; DMA out [128, 1024] tiles

Sharding: batch dim split 8 ways (2048 rows/core); small weights replicated.
"""

import numpy as np

import concourse.bass as bass
import concourse.mybir as mybir
from concourse import bacc
from concourse.tile import TileContext
from concourse.bass_utils import run_bass_kernel_spmd

N_CORES = 8
B_TOTAL = 16384
B_LOC = B_TOTAL // N_CORES  # 2048
P = 196  # 14*14 spatial features
V = 24  # views
F = P * V  # 4704 features per batch row
OUT = 1024
TILE = 128
G = 2  # 128-row tiles per group (stage-1 ops amortize instr overhead over G)
N_TILES = B_LOC // TILE  # 16
N_GROUPS = N_TILES // G  # 8
P2 = P - 128 + 1  # second contraction chunk: p 128..195 plus the ones row
PW = 512  # PSUM half width (max fp32 moving cols)

_cache = {}
last_results = None  # BassKernelResults of the most recent run (for test harness)


def _build_program(count_items):
    """count_items: tuple of (view, count) with count > 0."""
    nc = bacc.Bacc(
        "TRN2",
        target_bir_lowering=False,
        debug=False,
        num_devices=N_CORES,
    )
    f32 = mybir.dt.float32
    x_in = nc.declare_dram_parameter("x_in", [B_LOC, F], f32, isOutput=False)
    wxa = nc.declare_dram_parameter("wxa", [128, OUT], f32, isOutput=False)
    wxb = nc.declare_dram_parameter("wxb", [P2, OUT], f32, isOutput=False)
    ident = nc.declare_dram_parameter("ident", [128, 128], f32, isOutput=False)
    out = nc.declare_dram_parameter("out", [B_LOC, OUT], f32, isOutput=True)

    with TileContext(nc) as tc:
        with (
            tc.tile_pool(name="consts", bufs=1) as cpool,
            tc.tile_pool(name="x", bufs=3) as xpool,
            tc.tile_pool(name="acc", bufs=3) as apool,
            tc.tile_pool(name="lhs", bufs=4) as lpool,
            tc.tile_pool(name="outs", bufs=3) as opool,
            tc.tile_pool(name="pt", bufs=2, space="PSUM") as ptpool,
            tc.tile_pool(name="po", bufs=2, space="PSUM") as popool,
        ):
            wxa_t = cpool.tile([128, OUT], f32)
            nc.sync.dma_start(out=wxa_t[:], in_=wxa[:])
            wxb_t = cpool.tile([P2, OUT], f32)
            nc.sync.dma_start(out=wxb_t[:], in_=wxb[:])
            id_t = cpool.tile([128, 128], f32)
            nc.sync.dma_start(out=id_t[:], in_=ident[:])

            for grp in range(N_GROUPS):
                rows = slice(grp * G * TILE, (grp + 1) * G * TILE)
                xt = xpool.tile([TILE, G, F], f32)
                nc.sync.dma_start(
                    out=xt[:],
                    in_=x_in[rows, :].rearrange("(g p) f -> p g f", p=TILE),
                )

                acc = apool.tile([TILE, G, P + 1], f32)
                # ones column: folds const into the stage-2 contraction
                nc.vector.memset(acc[:, :, P : P + 1], 1.0)
                xv = xt.rearrange("p g (q v) -> p g q v", v=V)
                accs = acc[:, :, 0:P]
                for i, (v, c) in enumerate(count_items):
                    xs = xv[:, :, :, v]
                    if i == 0:
                        nc.vector.tensor_scalar_mul(accs, xs, float(c))
                    else:
                        nc.vector.scalar_tensor_tensor(
                            out=accs,
                            in0=xs,
                            scalar=float(c),
                            in1=accs,
                            op0=mybir.AluOpType.mult,
                            op1=mybir.AluOpType.add,
                        )

                ot = opool.tile([TILE, G, OUT], f32)
                for g in range(G):
                    pt1 = ptpool.tile([128, 128], f32, tag="pt1")
                    nc.tensor.transpose(pt1[:], acc[:, g, 0:128], id_t[:])
                    pt2 = ptpool.tile([P2, 128], f32, tag="pt2")
                    nc.tensor.transpose(pt2[:], acc[:, g, 128 : P + 1], id_t[:])
                    l1 = lpool.tile([128, 128], f32, tag="l1")
                    nc.scalar.copy(l1[:], pt1[:])
                    l2 = lpool.tile([P2, 128], f32, tag="l2")
                    nc.scalar.copy(l2[:], pt2[:])
                    po = popool.tile([128, OUT], f32)
                    for h in range(2):
                        cols = slice(h * PW, (h + 1) * PW)
                        nc.tensor.matmul(
                            po[:, cols], l1[:], wxa_t[:, cols], start=True, stop=False
                        )
                        nc.tensor.matmul(
                            po[:, cols], l2[:], wxb_t[:, cols], start=False, stop=True
                        )
                    nc.scalar.copy(ot[:, g, :], po[:])

                nc.sync.dma_start(
                    out=out[rows, :].rearrange("(g p) f -> p g f", p=TILE),
                    in_=ot[:],
                )
    nc.finalize()
    return nc


def kernel(x, indices, W_idx, b_idx, W_aggre, b_aggre, **run_kwargs):
    global last_results
    x = np.ascontiguousarray(np.asarray(x, dtype=np.float32)).reshape(B_TOTAL, F)
    idx = np.asarray(indices).astype(np.int64).ravel()
    W_idx = np.asarray(W_idx, dtype=np.float32)
    b_idx = np.asarray(b_idx, dtype=np.float32)
    W_aggre = np.asarray(W_aggre, dtype=np.float32)
    b_aggre = np.asarray(b_aggre, dtype=np.float32)

    count = np.bincount(idx, minlength=V)
    count_items = tuple((int(v), float(count[v])) for v in range(V) if count[v] != 0)

    # batch-independent index path, on host (tiny: [24, 196] scale)
    z = W_idx[:, idx].T + b_idx  # [K, 196]
    feats = np.where(z > 0, z, np.float32(0.2) * z).astype(np.float32)
    Wi = W_aggre[:, P:]
    const = feats.sum(0, dtype=np.float32) @ Wi.T + np.float32(len(idx)) * b_aggre

    WxT = np.ascontiguousarray(W_aggre[:, :P].T)  # [196, 1024]
    wxa_np = np.ascontiguousarray(WxT[:128])
    wxb_np = np.ascontiguousarray(
        np.concatenate([WxT[128:], const[None, :].astype(np.float32)], axis=0)
    )
    ident_np = np.eye(128, dtype=np.float32)

    nc = _cache.get(count_items)
    if nc is None:
        nc = _build_program(count_items)
        _cache[count_items] = nc

    in_maps = [
        {
            "x_in": np.ascontiguousarray(x[i * B_LOC : (i + 1) * B_LOC]),
            "wxa": wxa_np,
            "wxb": wxb_np,
            "ident": ident_np,
        }
        for i in range(N_CORES)
    ]
    res = run_bass_kernel_spmd(nc, in_maps, core_ids=list(range(N_CORES)), **run_kwargs)
    last_results = res
    return np.concatenate([res.results[i]["out"] for i in range(N_CORES)], axis=0)


# revision 15
# speedup vs baseline: 1.2746x; 1.0839x over previous
"""Trainium2 Bass kernel for nn_Aggregator (segment_reduce).

Math: the reference's gather+einsum collapses algebraically:
  out[b, o] = sum_p sum_k x[b, p, indices[k]] * Wx[o, p] + const[o]
            = sum_p (sum_v count[v] * x[b, p, v]) * Wx[o, p] + const[o]
where count = histogram of `indices` and const = index-path output + K*b_aggre
(both batch-independent, computed on host from the tiny weights).

Device pipeline per core (B_loc = 2048 rows, batch on partitions):
  1. DMA x tiles [128, G, 4704] (ramped group sizes; first loads chained to
     avoid prefetches racing the critical first tile)
  2. DVE: weighted view-reduction, one fused scalar_tensor_tensor per
     distinct view, split into q-halves so the strided read walks 192 B
     (2x faster per element than the natural 96 B stride)
  3. PE: transpose x_summed [128, 197] -> [197, 128] chunks (ones column
     folds the const row into the stage-2 contraction)
  4. PE: fp32 matmul [p, b]^T @ WxT[p, o] accumulating in PSUM
  5. ACT: PSUM -> SBUF copies; DMA out [128, 1024] tiles

Sharding: batch dim split 8 ways (2048 rows/core); small weights replicated.
"""

import numpy as np

import concourse.bass as bass
import concourse.mybir as mybir
from concourse import bacc
from concourse.tile import TileContext
from concourse.bass_utils import run_bass_kernel_spmd

N_CORES = 8
B_TOTAL = 16384
B_LOC = B_TOTAL // N_CORES  # 2048
P = 196  # 14*14 spatial features
HALF_Q = 98  # q-split: stage-1 APs walk q in two halves (stride 192 B)
V = 24  # views
F = P * V  # 4704 features per batch row
OUT = 1024
TILE = 128
N_TILES = B_LOC // TILE  # 16
GROUP_SIZES = [1, 1, 2, 2, 2, 2, 2, 2, 2]  # ramped; sums to 16
N_CHAINED_LOADS = 3  # serialize the first loads for fast pipeline start
P2 = P - 128 + 1  # second contraction chunk: p 128..195 plus the ones row
PW = 512  # PSUM half width (max fp32 moving cols)

_cache = {}
last_results = None  # BassKernelResults of the most recent run (for test harness)


def _build_program(count_items):
    """count_items: tuple of (view, count) with count > 0."""
    assert sum(GROUP_SIZES) == N_TILES
    nc = bacc.Bacc(
        "TRN2",
        target_bir_lowering=False,
        debug=False,
        num_devices=N_CORES,
    )
    f32 = mybir.dt.float32
    x_in = nc.declare_dram_parameter("x_in", [B_LOC, F], f32, isOutput=False)
    wxa = nc.declare_dram_parameter("wxa", [128, OUT], f32, isOutput=False)
    wxb = nc.declare_dram_parameter("wxb", [P2, OUT], f32, isOutput=False)
    ident = nc.declare_dram_parameter("ident", [128, 128], f32, isOutput=False)
    out = nc.declare_dram_parameter("out", [B_LOC, OUT], f32, isOutput=True)

    MU = mybir.AluOpType.mult
    AD = mybir.AluOpType.add

    with TileContext(nc) as tc:
        with (
            tc.tile_pool(name="consts", bufs=1) as cpool,
            tc.tile_pool(name="x", bufs=3) as xpool,
            tc.tile_pool(name="acc", bufs=3) as apool,
            tc.tile_pool(name="lhs", bufs=4) as lpool,
            tc.tile_pool(name="outs", bufs=3) as opool,
            tc.tile_pool(name="pt", bufs=2, space="PSUM") as ptpool,
            tc.tile_pool(name="po", bufs=2, space="PSUM") as popool,
        ):
            wxa_t = cpool.tile([128, OUT], f32)
            nc.sync.dma_start(out=wxa_t[:], in_=wxa[:])
            wxb_t = cpool.tile([P2, OUT], f32)
            nc.sync.dma_start(out=wxb_t[:], in_=wxb[:])
            id_t = cpool.tile([128, 128], f32)
            nc.sync.dma_start(out=id_t[:], in_=ident[:])

            row0 = 0
            for grp, G in enumerate(GROUP_SIZES):
                rows = slice(row0, row0 + G * TILE)
                row0 += G * TILE
                xt = xpool.tile([TILE, 2, F], f32, tag="xt")
                ld = nc.sync.dma_start(
                    out=xt[:, 0:G, :],
                    in_=x_in[rows, :].rearrange("(g p) f -> p g f", p=TILE),
                )
                if grp <= N_CHAINED_LOADS:
                    tc.chain_iter_dep("xload", ld.ins)

                acc = apool.tile([TILE, 2, P + 1], f32, tag="acc")
                # ones column: folds const into the stage-2 contraction
                nc.vector.memset(acc[:, 0:G, P : P + 1], 1.0)
                # stage-1 views with q split in halves: x walks 192 B strides
                xh = xt.rearrange("p g (q h v) -> p g q h v", v=V, h=2)
                ah = acc[:, :, 0:P].rearrange("p g (h q) -> p g h q", h=2)
                for i, (v, c) in enumerate(count_items):
                    for h in range(2):
                        xs = xh[:, 0:G, :, h, v]
                        dst = ah[:, 0:G, h, :]
                        if i == 0:
                            nc.vector.tensor_scalar_mul(dst, xs, float(c))
                        else:
                            nc.vector.scalar_tensor_tensor(
                                out=dst, in0=xs, scalar=float(c), in1=dst,
                                op0=MU, op1=AD,
                            )

                ot = opool.tile([TILE, 2, OUT], f32, tag="ot")
                for g in range(G):
                    pt1 = ptpool.tile([128, 128], f32, tag="pt1")
                    nc.tensor.transpose(pt1[:], acc[:, g, 0:128], id_t[:])
                    pt2 = ptpool.tile([P2, 128], f32, tag="pt2")
                    nc.tensor.transpose(pt2[:], acc[:, g, 128 : P + 1], id_t[:])
                    l1 = lpool.tile([128, 128], f32, tag="l1")
                    nc.scalar.copy(l1[:], pt1[:])
                    l2 = lpool.tile([P2, 128], f32, tag="l2")
                    nc.scalar.copy(l2[:], pt2[:])
                    po = popool.tile([128, OUT], f32)
                    for hh in range(2):
                        cols = slice(hh * PW, (hh + 1) * PW)
                        nc.tensor.matmul(
                            po[:, cols], l1[:], wxa_t[:, cols], start=True, stop=False
                        )
                        nc.tensor.matmul(
                            po[:, cols], l2[:], wxb_t[:, cols], start=False, stop=True
                        )
                    nc.scalar.copy(ot[:, g, :], po[:])

                nc.sync.dma_start(
                    out=out[rows, :].rearrange("(g p) f -> p g f", p=TILE),
                    in_=ot[:, 0:G, :],
                )
    nc.finalize()
    return nc


def kernel(x, indices, W_idx, b_idx, W_aggre, b_aggre, **run_kwargs):
    global last_results
    x = np.ascontiguousarray(np.asarray(x, dtype=np.float32)).reshape(B_TOTAL, F)
    idx = np.asarray(indices).astype(np.int64).ravel()
    W_idx = np.asarray(W_idx, dtype=np.float32)
    b_idx = np.asarray(b_idx, dtype=np.float32)
    W_aggre = np.asarray(W_aggre, dtype=np.float32)
    b_aggre = np.asarray(b_aggre, dtype=np.float32)

    count = np.bincount(idx, minlength=V)
    count_items = tuple((int(v), float(count[v])) for v in range(V) if count[v] != 0)

    # batch-independent index path, on host (tiny: [24, 196] scale)
    z = W_idx[:, idx].T + b_idx  # [K, 196]
    feats = np.where(z > 0, z, np.float32(0.2) * z).astype(np.float32)
    Wi = W_aggre[:, P:]
    const = feats.sum(0, dtype=np.float32) @ Wi.T + np.float32(len(idx)) * b_aggre

    WxT = np.ascontiguousarray(W_aggre[:, :P].T)  # [196, 1024]
    # stage-1 writes x_summed[p] to acc column j = (p%2)*98 + p//2 (q-split
    # interleave); permute WxT rows to match so the contraction stays aligned
    p_of_j = np.array(
        [2 * (j % HALF_Q) + j // HALF_Q for j in range(P)], dtype=np.int64
    )
    WxTp = WxT[p_of_j]
    wxa_np = np.ascontiguousarray(WxTp[:128])
    wxb_np = np.ascontiguousarray(
        np.concatenate([WxTp[128:], const[None, :].astype(np.float32)], axis=0)
    )
    ident_np = np.eye(128, dtype=np.float32)

    nc = _cache.get(count_items)
    if nc is None:
        nc = _build_program(count_items)
        _cache[count_items] = nc

    in_maps = [
        {
            "x_in": np.ascontiguousarray(x[i * B_LOC : (i + 1) * B_LOC]),
            "wxa": wxa_np,
            "wxb": wxb_np,
            "ident": ident_np,
        }
        for i in range(N_CORES)
    ]
    res = run_bass_kernel_spmd(nc, in_maps, core_ids=list(range(N_CORES)), **run_kwargs)
    last_results = res
    return np.concatenate([res.results[i]["out"] for i in range(N_CORES)], axis=0)


# revision 17
# speedup vs baseline: 1.2755x; 1.0008x over previous
"""Trainium2 Bass kernel for nn_Aggregator (segment_reduce).

Math: the reference's gather+einsum collapses algebraically:
  out[b, o] = sum_p sum_k x[b, p, indices[k]] * Wx[o, p] + const[o]
            = sum_p (sum_v count[v] * x[b, p, v]) * Wx[o, p] + const[o]
where count = histogram of `indices` and const = index-path output + K*b_aggre
(both batch-independent, computed on host from the tiny weights).

Device pipeline per core (B_loc = 2048 rows, batch on partitions):
  1. DMA x tiles [128, G, 4704] (ramped group sizes; first loads chained to
     avoid prefetches racing the critical first tile)
  2. DVE: weighted view-reduction, one fused scalar_tensor_tensor per
     distinct view, split into q-halves so the strided read walks 192 B
     (2x faster per element than the natural 96 B stride)
  3. PE: transpose x_summed [128, 197] -> [197, 128] chunks (ones column
     folds the const row into the stage-2 contraction)
  4. PE: fp32 matmul [p, b]^T @ WxT[p, o] accumulating in PSUM
  5. ACT: PSUM -> SBUF copies; DMA out [128, 1024] tiles

Sharding: batch dim split 8 ways (2048 rows/core); small weights replicated.
"""

import numpy as np

import concourse.bass as bass
import concourse.mybir as mybir
from concourse import bacc
from concourse.tile import TileContext
from concourse.bass_utils import run_bass_kernel_spmd

N_CORES = 8
B_TOTAL = 16384
B_LOC = B_TOTAL // N_CORES  # 2048
P = 196  # 14*14 spatial features
HALF_Q = 98  # q-split: stage-1 APs walk q in two halves (stride 192 B)
V = 24  # views
F = P * V  # 4704 features per batch row
OUT = 1024
TILE = 128
N_TILES = B_LOC // TILE  # 16
GROUP_SIZES = [1, 1, 2, 2, 2, 2, 2, 2, 1, 1]  # ramped both ends; sums to 16
P2 = P - 128 + 1  # second contraction chunk: p 128..195 plus the ones row
PW = 512  # PSUM half width (max fp32 moving cols)

_cache = {}
last_results = None  # BassKernelResults of the most recent run (for test harness)


def _build_program(count_items):
    """count_items: tuple of (view, count) with count > 0."""
    assert sum(GROUP_SIZES) == N_TILES
    nc = bacc.Bacc(
        "TRN2",
        target_bir_lowering=False,
        debug=False,
        num_devices=N_CORES,
    )
    f32 = mybir.dt.float32
    x_in = nc.declare_dram_parameter("x_in", [B_LOC, F], f32, isOutput=False)
    wxa = nc.declare_dram_parameter("wxa", [128, OUT], f32, isOutput=False)
    wxb = nc.declare_dram_parameter("wxb", [P2, OUT], f32, isOutput=False)
    ident = nc.declare_dram_parameter("ident", [128, 128], f32, isOutput=False)
    out = nc.declare_dram_parameter("out", [B_LOC, OUT], f32, isOutput=True)

    MU = mybir.AluOpType.mult
    AD = mybir.AluOpType.add

    with TileContext(nc) as tc:
        with (
            tc.tile_pool(name="consts", bufs=1) as cpool,
            tc.tile_pool(name="x", bufs=4) as xpool,
            tc.tile_pool(name="acc", bufs=3) as apool,
            tc.tile_pool(name="lhs", bufs=4) as lpool,
            tc.tile_pool(name="outs", bufs=2) as opool,
            tc.tile_pool(name="pt", bufs=2, space="PSUM") as ptpool,
            tc.tile_pool(name="po", bufs=2, space="PSUM") as popool,
        ):
            # issue the first x loads before the weight loads so the critical
            # first tile isn't queued behind them
            xtiles = []
            row0 = 0
            for grp, G in enumerate(GROUP_SIZES):
                rows = slice(row0, row0 + G * TILE)
                row0 += G * TILE
                if grp < 2:
                    xt = xpool.tile([TILE, 2, F], f32, tag="xt")
                    nc.sync.dma_start(
                        out=xt[:, 0:G, :],
                        in_=x_in[rows, :].rearrange("(g p) f -> p g f", p=TILE),
                    )
                    xtiles.append((xt, rows))
                else:
                    xtiles.append((None, rows))

            wxa_t = cpool.tile([128, OUT], f32)
            nc.sync.dma_start(out=wxa_t[:], in_=wxa[:])
            wxb_t = cpool.tile([P2, OUT], f32)
            nc.sync.dma_start(out=wxb_t[:], in_=wxb[:])
            id_t = cpool.tile([128, 128], f32)
            nc.sync.dma_start(out=id_t[:], in_=ident[:])

            for grp, G in enumerate(GROUP_SIZES):
                xt, rows = xtiles[grp]
                if xt is None:
                    xt = xpool.tile([TILE, 2, F], f32, tag="xt")
                    nc.sync.dma_start(
                        out=xt[:, 0:G, :],
                        in_=x_in[rows, :].rearrange("(g p) f -> p g f", p=TILE),
                    )

                acc = apool.tile([TILE, 2, P + 1], f32, tag="acc")
                # ones column: folds const into the stage-2 contraction
                nc.vector.memset(acc[:, 0:G, P : P + 1], 1.0)
                # stage-1 views with q split in halves: x walks 192 B strides
                xh = xt.rearrange("p g (q h v) -> p g q h v", v=V, h=2)
                ah = acc[:, :, 0:P].rearrange("p g (h q) -> p g h q", h=2)
                for i, (v, c) in enumerate(count_items):
                    for h in range(2):
                        xs = xh[:, 0:G, :, h, v]
                        dst = ah[:, 0:G, h, :]
                        if i == 0:
                            nc.vector.tensor_scalar_mul(dst, xs, float(c))
                        else:
                            nc.vector.scalar_tensor_tensor(
                                out=dst, in0=xs, scalar=float(c), in1=dst,
                                op0=MU, op1=AD,
                            )

                ot = opool.tile([TILE, 2, OUT], f32, tag="ot")
                for g in range(G):
                    pt1 = ptpool.tile([128, 128], f32, tag="pt1")
                    nc.tensor.transpose(pt1[:], acc[:, g, 0:128], id_t[:])
                    pt2 = ptpool.tile([P2, 128], f32, tag="pt2")
                    nc.tensor.transpose(pt2[:], acc[:, g, 128 : P + 1], id_t[:])
                    l1 = lpool.tile([128, 128], f32, tag="l1")
                    nc.scalar.copy(l1[:], pt1[:])
                    l2 = lpool.tile([P2, 128], f32, tag="l2")
                    nc.scalar.copy(l2[:], pt2[:])
                    po = popool.tile([128, OUT], f32)
                    for hh in range(2):
                        cols = slice(hh * PW, (hh + 1) * PW)
                        nc.tensor.matmul(
                            po[:, cols], l1[:], wxa_t[:, cols], start=True, stop=False
                        )
                        nc.tensor.matmul(
                            po[:, cols], l2[:], wxb_t[:, cols], start=False, stop=True
                        )
                    nc.scalar.copy(ot[:, g, :], po[:])

                nc.sync.dma_start(
                    out=out[rows, :].rearrange("(g p) f -> p g f", p=TILE),
                    in_=ot[:, 0:G, :],
                )
    nc.finalize()
    return nc


def kernel(x, indices, W_idx, b_idx, W_aggre, b_aggre, **run_kwargs):
    global last_results
    x = np.ascontiguousarray(np.asarray(x, dtype=np.float32)).reshape(B_TOTAL, F)
    idx = np.asarray(indices).astype(np.int64).ravel()
    W_idx = np.asarray(W_idx, dtype=np.float32)
    b_idx = np.asarray(b_idx, dtype=np.float32)
    W_aggre = np.asarray(W_aggre, dtype=np.float32)
    b_aggre = np.asarray(b_aggre, dtype=np.float32)

    count = np.bincount(idx, minlength=V)
    count_items = tuple((int(v), float(count[v])) for v in range(V) if count[v] != 0)

    # batch-independent index path, on host (tiny: [24, 196] scale)
    z = W_idx[:, idx].T + b_idx  # [K, 196]
    feats = np.where(z > 0, z, np.float32(0.2) * z).astype(np.float32)
    Wi = W_aggre[:, P:]
    const = feats.sum(0, dtype=np.float32) @ Wi.T + np.float32(len(idx)) * b_aggre

    WxT = np.ascontiguousarray(W_aggre[:, :P].T)  # [196, 1024]
    # stage-1 writes x_summed[p] to acc column j = (p%2)*98 + p//2 (q-split
    # interleave); permute WxT rows to match so the contraction stays aligned
    p_of_j = np.array(
        [2 * (j % HALF_Q) + j // HALF_Q for j in range(P)], dtype=np.int64
    )
    WxTp = WxT[p_of_j]
    wxa_np = np.ascontiguousarray(WxTp[:128])
    wxb_np = np.ascontiguousarray(
        np.concatenate([WxTp[128:], const[None, :].astype(np.float32)], axis=0)
    )
    ident_np = np.eye(128, dtype=np.float32)

    nc = _cache.get(count_items)
    if nc is None:
        nc = _build_program(count_items)
        _cache[count_items] = nc

    in_maps = [
        {
            "x_in": np.ascontiguousarray(x[i * B_LOC : (i + 1) * B_LOC]),
            "wxa": wxa_np,
            "wxb": wxb_np,
            "ident": ident_np,
        }
        for i in range(N_CORES)
    ]
    res = run_bass_kernel_spmd(nc, in_maps, core_ids=list(range(N_CORES)), **run_kwargs)
    last_results = res
    return np.concatenate([res.results[i]["out"] for i in range(N_CORES)], axis=0)


# revision 18
# speedup vs baseline: 1.3014x; 1.0203x over previous
"""Trainium2 Bass kernel for nn_Aggregator (segment_reduce).

Math: the reference's gather+einsum collapses algebraically:
  out[b, o] = sum_p sum_k x[b, p, indices[k]] * Wx[o, p] + const[o]
            = sum_p (sum_v count[v] * x[b, p, v]) * Wx[o, p] + const[o]
where count = histogram of `indices` and const = index-path output + K*b_aggre
(both batch-independent, computed on host from the tiny weights).

Device pipeline per core (B_loc = 2048 rows, batch on partitions, 16 tiles
of 128 rows, fully per-tile pipelined with deep DMA prefetch):
  1. DMA x tile [128, 4704]
  2. DVE: weighted view-reduction, one fused scalar_tensor_tensor per
     distinct view, split into q-halves so the strided read walks 192 B
     (2x faster per element than the natural 96 B stride)
  3. PE: transpose x_summed [128, 197] -> [197, 128] chunks (ones column
     folds the const row into the stage-2 contraction)
  4. PE: fp32 matmul [p, b]^T @ WxT[p, o] accumulating in PSUM
  5. ACT: PSUM -> SBUF copies; DMA out [128, 1024] tile

Sharding: batch dim split 8 ways (2048 rows/core); small weights replicated.
"""

import numpy as np

import concourse.bass as bass
import concourse.mybir as mybir
from concourse import bacc
from concourse.tile import TileContext
from concourse.bass_utils import run_bass_kernel_spmd

N_CORES = 8
B_TOTAL = 16384
B_LOC = B_TOTAL // N_CORES  # 2048
P = 196  # 14*14 spatial features
HALF_Q = 98  # q-split: stage-1 APs walk q in two halves (stride 192 B)
V = 24  # views
F = P * V  # 4704 features per batch row
OUT = 1024
TILE = 128
N_TILES = B_LOC // TILE  # 16
N_PRE = 2  # x loads issued ahead of the weight loads
P2 = P - 128 + 1  # second contraction chunk: p 128..195 plus the ones row
PW = 512  # PSUM half width (max fp32 moving cols)

_cache = {}
last_results = None  # BassKernelResults of the most recent run (for test harness)


def _build_program(count_items):
    """count_items: tuple of (view, count) with count > 0."""
    nc = bacc.Bacc(
        "TRN2",
        target_bir_lowering=False,
        debug=False,
        num_devices=N_CORES,
    )
    f32 = mybir.dt.float32
    x_in = nc.declare_dram_parameter("x_in", [B_LOC, F], f32, isOutput=False)
    wxa = nc.declare_dram_parameter("wxa", [128, OUT], f32, isOutput=False)
    wxb = nc.declare_dram_parameter("wxb", [P2, OUT], f32, isOutput=False)
    ident = nc.declare_dram_parameter("ident", [128, 128], f32, isOutput=False)
    out = nc.declare_dram_parameter("out", [B_LOC, OUT], f32, isOutput=True)

    MU = mybir.AluOpType.mult
    AD = mybir.AluOpType.add

    with TileContext(nc) as tc:
        with (
            tc.tile_pool(name="consts", bufs=1) as cpool,
            tc.tile_pool(name="x", bufs=8) as xpool,
            tc.tile_pool(name="acc", bufs=3) as apool,
            tc.tile_pool(name="lhs", bufs=4) as lpool,
            tc.tile_pool(name="outs", bufs=3) as opool,
            tc.tile_pool(name="pt", bufs=2, space="PSUM") as ptpool,
            tc.tile_pool(name="po", bufs=2, space="PSUM") as popool,
        ):
            # issue the first x loads before the weight loads so the critical
            # first tile isn't queued behind them
            xtiles = [None] * N_TILES

            def load(t):
                xt = xpool.tile([TILE, F], f32, tag="xt")
                nc.sync.dma_start(out=xt[:], in_=x_in[t * TILE : (t + 1) * TILE, :])
                xtiles[t] = xt

            for t in range(N_PRE):
                load(t)

            wxa_t = cpool.tile([128, OUT], f32)
            nc.sync.dma_start(out=wxa_t[:], in_=wxa[:])
            wxb_t = cpool.tile([P2, OUT], f32)
            nc.sync.dma_start(out=wxb_t[:], in_=wxb[:])
            id_t = cpool.tile([128, 128], f32)
            nc.sync.dma_start(out=id_t[:], in_=ident[:])

            for t in range(N_TILES):
                if xtiles[t] is None:
                    load(t)
                xt = xtiles[t]

                acc = apool.tile([TILE, P + 1], f32, tag="acc")
                # ones column: folds const into the stage-2 contraction
                nc.vector.memset(acc[:, P : P + 1], 1.0)
                # stage-1 views with q split in halves: x walks 192 B strides
                xh = xt.rearrange("p (q h v) -> p q h v", v=V, h=2)
                ah = acc[:, 0:P].rearrange("p (h q) -> p h q", h=2)
                for i, (v, c) in enumerate(count_items):
                    for h in range(2):
                        xs = xh[:, :, h, v]
                        dst = ah[:, h, :]
                        if i == 0:
                            nc.vector.tensor_scalar_mul(dst, xs, float(c))
                        else:
                            nc.vector.scalar_tensor_tensor(
                                out=dst, in0=xs, scalar=float(c), in1=dst,
                                op0=MU, op1=AD,
                            )

                pt1 = ptpool.tile([128, 128], f32, tag="pt1")
                nc.tensor.transpose(pt1[:], acc[:, 0:128], id_t[:])
                pt2 = ptpool.tile([P2, 128], f32, tag="pt2")
                nc.tensor.transpose(pt2[:], acc[:, 128 : P + 1], id_t[:])
                l1 = lpool.tile([128, 128], f32, tag="l1")
                nc.scalar.copy(l1[:], pt1[:])
                l2 = lpool.tile([P2, 128], f32, tag="l2")
                nc.scalar.copy(l2[:], pt2[:])
                po = popool.tile([128, OUT], f32)
                for hh in range(2):
                    cols = slice(hh * PW, (hh + 1) * PW)
                    nc.tensor.matmul(
                        po[:, cols], l1[:], wxa_t[:, cols], start=True, stop=False
                    )
                    nc.tensor.matmul(
                        po[:, cols], l2[:], wxb_t[:, cols], start=False, stop=True
                    )
                ot = opool.tile([TILE, OUT], f32, tag="ot")
                nc.scalar.copy(ot[:], po[:])
                nc.sync.dma_start(out=out[t * TILE : (t + 1) * TILE, :], in_=ot[:])
    nc.finalize()
    return nc


def kernel(x, indices, W_idx, b_idx, W_aggre, b_aggre, **run_kwargs):
    global last_results
    x = np.ascontiguousarray(np.asarray(x, dtype=np.float32)).reshape(B_TOTAL, F)
    idx = np.asarray(indices).astype(np.int64).ravel()
    W_idx = np.asarray(W_idx, dtype=np.float32)
    b_idx = np.asarray(b_idx, dtype=np.float32)
    W_aggre = np.asarray(W_aggre, dtype=np.float32)
    b_aggre = np.asarray(b_aggre, dtype=np.float32)

    count = np.bincount(idx, minlength=V)
    count_items = tuple((int(v), float(count[v])) for v in range(V) if count[v] != 0)

    # batch-independent index path, on host (tiny: [24, 196] scale)
    z = W_idx[:, idx].T + b_idx  # [K, 196]
    feats = np.where(z > 0, z, np.float32(0.2) * z).astype(np.float32)
    Wi = W_aggre[:, P:]
    const = feats.sum(0, dtype=np.float32) @ Wi.T + np.float32(len(idx)) * b_aggre

    WxT = np.ascontiguousarray(W_aggre[:, :P].T)  # [196, 1024]
    # stage-1 writes x_summed[p] to acc column j = (p%2)*98 + p//2 (q-split
    # interleave); permute WxT rows to match so the contraction stays aligned
    p_of_j = np.array(
        [2 * (j % HALF_Q) + j // HALF_Q for j in range(P)], dtype=np.int64
    )
    WxTp = WxT[p_of_j]
    wxa_np = np.ascontiguousarray(WxTp[:128])
    wxb_np = np.ascontiguousarray(
        np.concatenate([WxTp[128:], const[None, :].astype(np.float32)], axis=0)
    )
    ident_np = np.eye(128, dtype=np.float32)

    nc = _cache.get(count_items)
    if nc is None:
        nc = _build_program(count_items)
        _cache[count_items] = nc

    in_maps = [
        {
            "x_in": np.ascontiguousarray(x[i * B_LOC : (i + 1) * B_LOC]),
            "wxa": wxa_np,
            "wxb": wxb_np,
            "ident": ident_np,
        }
        for i in range(N_CORES)
    ]
    res = run_bass_kernel_spmd(nc, in_maps, core_ids=list(range(N_CORES)), **run_kwargs)
    last_results = res
    return np.concatenate([res.results[i]["out"] for i in range(N_CORES)], axis=0)
